# revision 17
# baseline (speedup 1.0000x reference)
"""Trainium2 Bass kernel for single-head attention (no mask).

Reference computation (B=4, S=2048, D=1024):
    q = x @ Wq.T ; k = x @ Wk.T ; v = x @ Wv.T          (per batch)
    out = softmax((q @ k.T) / sqrt(1024)) @ v

Sharding: 8 cores = (batch, query-half); each core computes its 1024
query rows against the full 2048-key sequence of its batch (attention
is invariant to the common row permutation that puts the core's query
half first). No collectives (a pair K/V exchange costs ~225us in the
measured-collective model vs ~55us of PE it would save).

Algebraic structure (keys/values never materialized):
    scores   = q k^T = x (Wq^T Wk) x^T      with M = Wq^T Wk from host
    out      = softmax(scores) (x Wv^T) = (softmax(scores) x) Wv^T
so the device work is four dense stages, all operands bf16 in SBUF:
    A: H   = M^T x_q^T                 [d,  q]   65.5k PE cycles
    B: S^T = x H   -> exp -> E^T       [k,  q]  131k (+16.4k rowsum)
    C: C^T = x^T E^T  (/rowsum)        [d,  q]  131k
    D: o^T = Wv C^T                    [o,  q]   65.5k
vs ~648k cycles for the direct QKV formulation: projections shrink to
the 1024-wide post-softmax contraction, M/Wv^T/x^T/x are host-prepped.

The emission order keeps the PE stream gap-free (the cost model resets
the clock-ramp p-state on every idle gap): a warmup matmul chain holds
the PE from t~0.4us until the first loads land, phase A runs 8 chains
interleaved (borrowing the 2 rowsum PSUM banks) so consumption stays
behind the single-queue DMA feed, chain finishes are staggered with
evictions split across DVE+Act, exp/rowsum interleave one key-tile
behind the scores, and the final rowsum + reciprocal hide inside phase
C's first chain.
"""

import ml_dtypes
import numpy as np

import concourse.bass as bass
import concourse.tile as tile
from concourse import bacc, bass_isa, mybir
from concourse.bass_utils import run_bass_kernel_spmd

B, S, D, O = 4, 2048, 1024, 1024
HQ = S // 2  # query rows per core
N_CORES = 8
BF = mybir.dt.bfloat16
F32 = mybir.dt.float32
SCALE = 1.0 / 32.0  # 1/sqrt(1024)
DK = D // 128  # 8 contraction tiles over d
KT = S // 128  # 16 key tiles
OT = O // 128  # 8 output o-tiles
NWARM = 17  # warmup matmuls bridging the initial DMA latency

_CACHE: dict = {}


def _emit(nc, sfx=""):
    xt_d = nc.dram_tensor(f"xt{sfx}", [D, S], BF, kind="ExternalInput")
    xn_d = nc.dram_tensor(f"xn{sfx}", [S, D], BF, kind="ExternalInput")
    m_d = nc.dram_tensor(f"m{sfx}", [D, D], BF, kind="ExternalInput")
    wvt_d = nc.dram_tensor(f"wvt{sfx}", [D, O], BF, kind="ExternalInput")
    out_d = nc.dram_tensor(f"outT{sfx}", [O, HQ], F32, kind="ExternalOutput")

    with tile.TileContext(nc) as tc:
        with (
            tc.tile_pool(name=f"{sfx}sb", bufs=1) as sb,
            tc.tile_pool(name=f"{sfx}pp", bufs=6, space="PSUM") as pp,
            tc.tile_pool(name=f"{sfx}rs", bufs=2, space="PSUM") as rs,
        ):
            # Per-block tiles so dependency tracking stays fine-grained.
            mt = [sb.tile([128, D], BF, tag=f"mt{i}", name=f"mt{sfx}_{i}") for i in range(1, DK)]
            mt.insert(0, None)
            m0l = sb.tile([128, 512], BF, tag="m0l", name=f"m0l{sfx}")
            m0h = sb.tile([128, 512], BF, tag="m0h", name=f"m0h{sfx}")
            xq = [sb.tile([128, HQ], BF, tag=f"xq{i}", name=f"xq{sfx}_{i}") for i in range(1, DK)]
            xq.insert(0, None)
            xq0l = sb.tile([128, 512], BF, tag="xq0l", name=f"xq0l{sfx}")
            xq0h = sb.tile([128, 512], BF, tag="xq0h", name=f"xq0h{sfx}")
            xk = [sb.tile([128, HQ], BF, tag=f"xk{i}", name=f"xk{sfx}_{i}") for i in range(DK)]
            xn = [sb.tile([128, D], BF, tag=f"xn{i}", name=f"xn{sfx}_{i}") for i in range(KT)]
            wv = [sb.tile([128, O], BF, tag=f"wv{i}", name=f"wv{sfx}_{i}") for i in range(DK)]
            ht = [sb.tile([128, HQ], BF, tag=f"ht{i}", name=f"ht{sfx}_{i}") for i in range(DK)]
            et = [sb.tile([128, HQ], BF, tag=f"et{i}", name=f"et{sfx}_{i}") for i in range(KT)]
            ct = [sb.tile([128, HQ], BF, tag=f"ct{i}", name=f"ct{sfx}_{i}") for i in range(DK)]
            onest = sb.tile([128, 128], BF, tag="ones", name=f"onest{sfx}")
            warmt = sb.tile([128, 128], BF, tag="warm", name=f"warmt{sfx}")
            recip = sb.tile([128, HQ], F32, tag="recip", name=f"recip{sfx}")
            rowacc = sb.tile([128, HQ], F32, tag="rowacc", name=f"rowacc{sfx}")

            # Constants via memset (no DMA bandwidth). warmt on DVE so
            # the warmup chain can start early.
            nc.vector.memset(warmt, 0.0)
            nc.gpsimd.memset(onest, 1.0)
            # Dummy exp: hoists the Act engine's one-time activation-table
            # load into the idle startup window (it otherwise lands right
            # before the phase-A hi-evictions and delays them).
            actwarm = sb.tile([128, 1], BF, tag="actwarm", name=f"actwarm{sfx}")
            nc.scalar.activation(
                out=actwarm,
                in_=onest[:, 0:1],
                func=mybir.ActivationFunctionType.Exp,
                scale=SCALE,
            )

            # ---- DMA loads: one in-order queue = explicit priority ----
            # (mtl_i, xq_i, mth_i) triplets feed phase A; everything later
            # is needed tens of us after it lands.
            nc.sync.dma_start(out=m0l, in_=m_d[0:128, 0:512])
            nc.sync.dma_start(out=xq0l, in_=xt_d[0:128, 0:512])
            nc.sync.dma_start(out=xq0h, in_=xt_d[0:128, 512:HQ])
            nc.sync.dma_start(out=m0h, in_=m_d[0:128, 512:D])
            for i in range(1, DK):
                nc.sync.dma_start(out=mt[i], in_=m_d[i * 128 : (i + 1) * 128, :])
                nc.sync.dma_start(out=xq[i], in_=xt_d[i * 128 : (i + 1) * 128, 0:HQ])
            for i in range(DK):
                nc.sync.dma_start(out=xk[i], in_=xt_d[i * 128 : (i + 1) * 128, HQ:S])
            for i in range(DK):
                nc.sync.dma_start(out=wv[i], in_=wvt_d[i * 128 : (i + 1) * 128, :])
            for i in range(KT):
                nc.sync.dma_start(out=xn[i], in_=xn_d[i * 128 : (i + 1) * 128, :])

            # ---- PE warmup: hold the p-state until the first loads land ----
            wps = pp.tile([128, 512], F32, tag="ps", name=f"wps{sfx}")
            for i in range(NWARM):
                nc.tensor.matmul(wps[:, 0:128], warmt, warmt, start=True, stop=True)

            # ---- Phase A: H = M^T x_q^T ----
            a_ps = {}

            def a_mm(ch, d1t):
                d2t, qc = divmod(ch, 2)
                if d1t == 0:
                    stat = (m0l if d2t < 4 else m0h)[
                        :, (d2t % 4) * 128 : (d2t % 4 + 1) * 128
                    ]
                    mov = xq0l if qc == 0 else xq0h
                else:
                    stat = mt[d1t][:, d2t * 128 : (d2t + 1) * 128]
                    mov = xq[d1t][:, qc * 512 : (qc + 1) * 512]
                nc.tensor.matmul(
                    a_ps[ch],
                    stat,
                    mov,
                    start=(d1t == 0),
                    stop=(d1t == DK - 1),
                )

            def a_evict(ch):
                # Alternate DVE / Act per chain so eviction keeps pace with
                # the PE and the first eviction lands as early as possible.
                d2t, qc = divmod(ch, 2)
                dst = ht[d2t][:, qc * 512 : (qc + 1) * 512]
                if ch % 2 == 0:
                    nc.vector.tensor_copy(out=dst, in_=a_ps[ch])
                else:
                    nc.scalar.copy(out=dst, in_=a_ps[ch])

            # Two 8-chain interleaved waves: PE consumes one (mtl, xq, mth)
            # DMA triplet per 8 matmuls, slower than the DMA feed; chain
            # finishes are staggered so evictions free banks early.
            for wave in range(2):
                lo = 8 * wave
                for j, ch in enumerate(range(lo, lo + 8)):
                    pool = pp if j < 6 else rs
                    tag = "ps" if j < 6 else "rs"
                    a_ps[ch] = pool.tile(
                        [128, 512], F32, tag=tag, name=f"aps{sfx}_{ch}"
                    )
                for d1t in range(DK - 1):
                    for ch in range(lo, lo + 8):
                        a_mm(ch, d1t)
                for ch in range(lo, lo + 8):
                    a_mm(ch, DK - 1)
                    a_evict(ch)

            # ---- Phase B: S^T = x H, exp, rowsums ----
            # Rowsums run entirely off-PE: Pool partition-reduces each
            # exp'd key tile, DVE accumulates across tiles.
            def rowsum(kt):
                for qc in range(2):
                    rtmp = sb.tile(
                        [128, 512], F32, tag=f"rtmp{qc}", bufs=2,
                        name=f"rtmp{sfx}_{kt}_{qc}",
                    )
                    nc.gpsimd.partition_all_reduce(
                        rtmp,
                        et[kt][:, qc * 512 : (qc + 1) * 512],
                        128,
                        bass_isa.ReduceOp.add,
                    )
                    if kt == 0:
                        nc.vector.tensor_copy(
                            out=rowacc[:, qc * 512 : (qc + 1) * 512], in_=rtmp
                        )
                    else:
                        nc.vector.scalar_tensor_tensor(
                            out=rowacc[:, qc * 512 : (qc + 1) * 512],
                            in0=rtmp,
                            scalar=0.0,
                            in1=rowacc[:, qc * 512 : (qc + 1) * 512],
                            op0=mybir.AluOpType.bypass,
                            op1=mybir.AluOpType.add,
                        )

            def xq_slice(d2t, ki):
                # x^T query-half block d2t, key columns [ki*128, (ki+1)*128)
                if d2t == 0:
                    src = xq0l if ki < 4 else xq0h
                    return src[:, (ki % 4) * 128 : (ki % 4 + 1) * 128]
                return xq[d2t][:, ki * 128 : (ki + 1) * 128]

            for kt in range(KT):
                for qc in range(2):
                    sp = pp.tile([128, 512], F32, tag="ps", name=f"sps{sfx}_{kt}_{qc}")
                    for d2t in range(DK):
                        stat = (
                            xq_slice(d2t, kt)
                            if kt < DK
                            else xk[d2t][:, (kt - DK) * 128 : (kt - DK + 1) * 128]
                        )
                        nc.tensor.matmul(
                            sp,
                            stat,
                            ht[d2t][:, qc * 512 : (qc + 1) * 512],
                            start=(d2t == 0),
                            stop=(d2t == DK - 1),
                        )
                    nc.scalar.activation(
                        out=et[kt][:, qc * 512 : (qc + 1) * 512],
                        in_=sp,
                        func=mybir.ActivationFunctionType.Exp,
                        scale=SCALE,
                    )
                rowsum(kt)

            # ---- Phase C: C^T = x^T E^T, normalized at eviction ----
            for ch in range(16):
                dt, qc = divmod(ch, 2)
                c_ps = pp.tile([128, 512], F32, tag="ps", name=f"cps{sfx}_{ch}")
                for kt in range(KT):
                    nc.tensor.matmul(
                        c_ps,
                        xn[kt][:, dt * 128 : (dt + 1) * 128],
                        et[kt][:, qc * 512 : (qc + 1) * 512],
                        start=(kt == 0),
                        stop=(kt == KT - 1),
                    )
                if ch == 0:
                    for qc2 in range(2):
                        nc.vector.reciprocal(
                            out=recip[:, qc2 * 512 : (qc2 + 1) * 512],
                            in_=rowacc[:, qc2 * 512 : (qc2 + 1) * 512],
                        )
                nc.vector.scalar_tensor_tensor(
                    out=ct[dt][:, qc * 512 : (qc + 1) * 512],
                    in0=c_ps,
                    scalar=0.0,
                    in1=recip[:, qc * 512 : (qc + 1) * 512],
                    op0=mybir.AluOpType.bypass,
                    op1=mybir.AluOpType.mult,
                )

            # ---- Phase D: out^T = Wv C^T ----
            for ch in range(15):
                ot, qc = divmod(ch, 2)
                d_ps = pp.tile([128, 512], F32, tag="ps", name=f"dps{sfx}_{ch}")
                for dk in range(DK):
                    nc.tensor.matmul(
                        d_ps,
                        wv[dk][:, ot * 128 : (ot + 1) * 128],
                        ct[dk][:, qc * 512 : (qc + 1) * 512],
                        start=(dk == 0),
                        stop=(dk == DK - 1),
                    )
                oev = sb.tile(
                    [128, 512], F32, tag="oev", bufs=3, name=f"oev{sfx}_{ch}"
                )
                nc.vector.tensor_copy(out=oev, in_=d_ps)
                dma_eng = nc.sync if ch % 2 == 0 else nc.gpsimd
                dma_eng.dma_start(
                    out=out_d[ot * 128 : (ot + 1) * 128, qc * 512 : (qc + 1) * 512],
                    in_=oev,
                )
            # Final (ot7, qc1) chain as 4 [128,128] sub-chains so the tail
            # is one narrow evict+DMA instead of a full 512-wide one.
            for c4 in range(4):
                f_ps = pp.tile([128, 512], F32, tag="ps", name=f"fps{sfx}_{c4}")[
                    :, 0:128
                ]
                lo = 512 + c4 * 128
                for dk in range(DK):
                    nc.tensor.matmul(
                        f_ps,
                        wv[dk][:, 7 * 128 : 8 * 128],
                        ct[dk][:, lo : lo + 128],
                        start=(dk == 0),
                        stop=(dk == DK - 1),
                    )
                fev = sb.tile([128, 128], F32, tag="fev", bufs=4, name=f"fev{sfx}_{c4}")
                if c4 % 2 == 0:
                    nc.vector.tensor_copy(out=fev, in_=f_ps)
                else:
                    nc.scalar.copy(out=fev, in_=f_ps)
                dma_eng = nc.gpsimd if c4 % 2 == 0 else nc.sync
                dma_eng.dma_start(
                    out=out_d[7 * 128 : 8 * 128, lo : lo + 128], in_=fev
                )
    return nc


def _get_program():
    if "nc" not in _CACHE:
        nc = bacc.Bacc("TRN2", target_bir_lowering=False, num_devices=N_CORES)
        _emit(nc)
        nc.compile()
        _CACHE["nc"] = nc
    return _CACHE["nc"]


def kernel(x, Wq, Wk, Wv):
    bf = ml_dtypes.bfloat16
    x = np.asarray(x, dtype=np.float32)
    Wq = np.asarray(Wq, dtype=np.float32)
    Wk = np.asarray(Wk, dtype=np.float32)
    Wv = np.asarray(Wv, dtype=np.float32)

    nc = _get_program()
    m = np.ascontiguousarray(Wq.T @ Wk).astype(bf)  # M = Wq^T Wk, [d1, d2]
    wvt = np.ascontiguousarray(Wv.T).astype(bf)  # [D, O]
    in_maps = []
    for c in range(N_CORES):
        b, h = divmod(c, 2)
        xp = np.concatenate(
            [x[b, h * HQ : (h + 1) * HQ], x[b, (1 - h) * HQ : (2 - h) * HQ]], axis=0
        )
        in_maps.append(
            {
                "xt": np.ascontiguousarray(xp.T).astype(bf),
                "xn": xp.astype(bf),
                "m": m,
                "wvt": wvt,
            }
        )
    res = run_bass_kernel_spmd(nc, in_maps, list(range(N_CORES)))
    outp = np.empty((B, S, O), dtype=np.float32)
    for c in range(N_CORES):
        b, h = divmod(c, 2)
        outp[b, h * HQ : (h + 1) * HQ] = res.results[c]["outT"].T
    return outp


# revision 18
# speedup vs baseline: 1.0025x; 1.0025x over previous
"""Trainium2 Bass kernel for single-head attention (no mask).

Reference computation (B=4, S=2048, D=1024):
    q = x @ Wq.T ; k = x @ Wk.T ; v = x @ Wv.T          (per batch)
    out = softmax((q @ k.T) / sqrt(1024)) @ v

Sharding: 8 cores = (batch, query-half); each core computes its 1024
query rows against the full 2048-key sequence of its batch (attention
is invariant to the common row permutation that puts the core's query
half first). No collectives (a pair K/V exchange costs ~225us in the
measured-collective model vs ~55us of PE it would save).

Algebraic structure (keys/values never materialized):
    scores   = q k^T = x (Wq^T Wk) x^T      with M = Wq^T Wk from host
    out      = softmax(scores) (x Wv^T) = (softmax(scores) x) Wv^T
so the device work is four dense stages, all operands bf16 in SBUF:
    A: H   = M^T x_q^T                 [d,  q]   65.5k PE cycles
    B: S^T = x H   -> exp -> E^T       [k,  q]  131k (+16.4k rowsum)
    C: C^T = x^T E^T  (/rowsum)        [d,  q]  131k
    D: o^T = Wv C^T                    [o,  q]   65.5k
vs ~648k cycles for the direct QKV formulation: projections shrink to
the 1024-wide post-softmax contraction, M/Wv^T/x^T/x are host-prepped.

The emission order keeps the PE stream gap-free (the cost model resets
the clock-ramp p-state on every idle gap): a warmup matmul chain holds
the PE from t~0.4us until the first loads land, phase A runs 8 chains
interleaved (borrowing the 2 rowsum PSUM banks) so consumption stays
behind the single-queue DMA feed, chain finishes are staggered with
evictions split across DVE+Act, exp/rowsum interleave one key-tile
behind the scores, and the final rowsum + reciprocal hide inside phase
C's first chain.
"""

import ml_dtypes
import numpy as np

import concourse.bass as bass
import concourse.tile as tile
from concourse import bacc, bass_isa, mybir
from concourse.bass_utils import run_bass_kernel_spmd

B, S, D, O = 4, 2048, 1024, 1024
HQ = S // 2  # query rows per core
N_CORES = 8
BF = mybir.dt.bfloat16
F32 = mybir.dt.float32
SCALE = 1.0 / 32.0  # 1/sqrt(1024)
DK = D // 128  # 8 contraction tiles over d
KT = S // 128  # 16 key tiles
OT = O // 128  # 8 output o-tiles
NWARM = 31  # warmup matmuls bridging the initial DMA latency

_CACHE: dict = {}


def _emit(nc, sfx=""):
    xt_d = nc.dram_tensor(f"xt{sfx}", [D, S], BF, kind="ExternalInput")
    xn_d = nc.dram_tensor(f"xn{sfx}", [S, D], BF, kind="ExternalInput")
    m_d = nc.dram_tensor(f"m{sfx}", [D, D], BF, kind="ExternalInput")
    wvt_d = nc.dram_tensor(f"wvt{sfx}", [D, O], BF, kind="ExternalInput")
    out_d = nc.dram_tensor(f"outT{sfx}", [O, HQ], F32, kind="ExternalOutput")

    with tile.TileContext(nc) as tc:
        with (
            tc.tile_pool(name=f"{sfx}sb", bufs=1) as sb,
            tc.tile_pool(name=f"{sfx}pp", bufs=6, space="PSUM") as pp,
            tc.tile_pool(name=f"{sfx}rs", bufs=2, space="PSUM") as rs,
        ):
            # Per-block tiles so dependency tracking stays fine-grained.
            mt = [sb.tile([128, D], BF, tag=f"mt{i}", name=f"mt{sfx}_{i}") for i in range(DK)]
            xq = [sb.tile([128, HQ], BF, tag=f"xq{i}", name=f"xq{sfx}_{i}") for i in range(DK)]
            xk = [sb.tile([128, HQ], BF, tag=f"xk{i}", name=f"xk{sfx}_{i}") for i in range(DK)]
            xn = [sb.tile([128, D], BF, tag=f"xn{i}", name=f"xn{sfx}_{i}") for i in range(KT)]
            wv = [sb.tile([128, O], BF, tag=f"wv{i}", name=f"wv{sfx}_{i}") for i in range(DK)]
            ht = [sb.tile([128, HQ], BF, tag=f"ht{i}", name=f"ht{sfx}_{i}") for i in range(DK)]
            et = [sb.tile([128, HQ], BF, tag=f"et{i}", name=f"et{sfx}_{i}") for i in range(KT)]
            ct = [sb.tile([128, HQ], BF, tag=f"ct{i}", name=f"ct{sfx}_{i}") for i in range(DK)]
            onest = sb.tile([128, 128], BF, tag="ones", name=f"onest{sfx}")
            warmt = sb.tile([128, 128], BF, tag="warm", name=f"warmt{sfx}")
            recip = sb.tile([128, HQ], F32, tag="recip", name=f"recip{sfx}")
            rowacc = sb.tile([128, HQ], F32, tag="rowacc", name=f"rowacc{sfx}")

            # Constants via memset (no DMA bandwidth). warmt on DVE so
            # the warmup chain can start early.
            nc.vector.memset(warmt, 0.0)
            nc.gpsimd.memset(onest, 1.0)
            # Dummy exp: hoists the Act engine's one-time activation-table
            # load into the idle startup window (it otherwise lands right
            # before the phase-A hi-evictions and delays them).
            actwarm = sb.tile([128, 1], BF, tag="actwarm", name=f"actwarm{sfx}")
            nc.scalar.activation(
                out=actwarm,
                in_=onest[:, 0:1],
                func=mybir.ActivationFunctionType.Exp,
                scale=SCALE,
            )

            # ---- DMA loads: one in-order queue = explicit priority ----
            # (mtl_i, xq_i, mth_i) triplets feed phase A; everything later
            # is needed tens of us after it lands.
            for i in range(DK):
                nc.sync.dma_start(out=mt[i], in_=m_d[i * 128 : (i + 1) * 128, :])
                nc.sync.dma_start(out=xq[i], in_=xt_d[i * 128 : (i + 1) * 128, 0:HQ])
            for i in range(DK):
                nc.sync.dma_start(out=xk[i], in_=xt_d[i * 128 : (i + 1) * 128, HQ:S])
            for i in range(DK):
                nc.sync.dma_start(out=wv[i], in_=wvt_d[i * 128 : (i + 1) * 128, :])
            for i in range(KT):
                nc.sync.dma_start(out=xn[i], in_=xn_d[i * 128 : (i + 1) * 128, :])

            # ---- PE warmup: hold the p-state until the first loads land ----
            wps = pp.tile([128, 512], F32, tag="ps", name=f"wps{sfx}")
            for i in range(NWARM):
                nc.tensor.matmul(wps[:, 0:128], warmt, warmt, start=True, stop=True)

            # ---- Phase A: H = M^T x_q^T ----
            a_ps = {}

            def a_mm(ch, d1t):
                d2t, qc = divmod(ch, 2)
                nc.tensor.matmul(
                    a_ps[ch],
                    mt[d1t][:, d2t * 128 : (d2t + 1) * 128],
                    xq[d1t][:, qc * 512 : (qc + 1) * 512],
                    start=(d1t == 0),
                    stop=(d1t == DK - 1),
                )

            def a_evict(ch):
                # Alternate DVE / Act per chain so eviction keeps pace with
                # the PE and the first eviction lands as early as possible.
                d2t, qc = divmod(ch, 2)
                dst = ht[d2t][:, qc * 512 : (qc + 1) * 512]
                if ch % 2 == 0:
                    nc.vector.tensor_copy(out=dst, in_=a_ps[ch])
                else:
                    nc.scalar.copy(out=dst, in_=a_ps[ch])

            # Two 8-chain interleaved waves: PE consumes one (mtl, xq, mth)
            # DMA triplet per 8 matmuls, slower than the DMA feed; chain
            # finishes are staggered so evictions free banks early.
            for wave in range(2):
                lo = 8 * wave
                for j, ch in enumerate(range(lo, lo + 8)):
                    pool = pp if j < 6 else rs
                    tag = "ps" if j < 6 else "rs"
                    a_ps[ch] = pool.tile(
                        [128, 512], F32, tag=tag, name=f"aps{sfx}_{ch}"
                    )
                for d1t in range(DK - 1):
                    for ch in range(lo, lo + 8):
                        a_mm(ch, d1t)
                for ch in range(lo, lo + 8):
                    a_mm(ch, DK - 1)
                    a_evict(ch)

            # ---- Phase B: S^T = x H, exp, rowsums ----
            # Rowsums run entirely off-PE: Pool partition-reduces each
            # exp'd key tile, DVE accumulates across tiles.
            def rowsum(kt):
                for qc in range(2):
                    rtmp = sb.tile(
                        [128, 512], F32, tag=f"rtmp{qc}", bufs=2,
                        name=f"rtmp{sfx}_{kt}_{qc}",
                    )
                    nc.gpsimd.partition_all_reduce(
                        rtmp,
                        et[kt][:, qc * 512 : (qc + 1) * 512],
                        128,
                        bass_isa.ReduceOp.add,
                    )
                    if kt == 0:
                        nc.vector.tensor_copy(
                            out=rowacc[:, qc * 512 : (qc + 1) * 512], in_=rtmp
                        )
                    else:
                        nc.vector.scalar_tensor_tensor(
                            out=rowacc[:, qc * 512 : (qc + 1) * 512],
                            in0=rtmp,
                            scalar=0.0,
                            in1=rowacc[:, qc * 512 : (qc + 1) * 512],
                            op0=mybir.AluOpType.bypass,
                            op1=mybir.AluOpType.add,
                        )

            for kt in range(KT):
                xsrc, ki = (xq, kt) if kt < DK else (xk, kt - DK)
                for qc in range(2):
                    sp = pp.tile([128, 512], F32, tag="ps", name=f"sps{sfx}_{kt}_{qc}")
                    for d2t in range(DK):
                        nc.tensor.matmul(
                            sp,
                            xsrc[d2t][:, ki * 128 : (ki + 1) * 128],
                            ht[d2t][:, qc * 512 : (qc + 1) * 512],
                            start=(d2t == 0),
                            stop=(d2t == DK - 1),
                        )
                    nc.scalar.activation(
                        out=et[kt][:, qc * 512 : (qc + 1) * 512],
                        in_=sp,
                        func=mybir.ActivationFunctionType.Exp,
                        scale=SCALE,
                    )
                rowsum(kt)

            # ---- Phase C: C^T = x^T E^T, normalized at eviction ----
            for ch in range(16):
                dt, qc = divmod(ch, 2)
                c_ps = pp.tile([128, 512], F32, tag="ps", name=f"cps{sfx}_{ch}")
                for kt in range(KT):
                    nc.tensor.matmul(
                        c_ps,
                        xn[kt][:, dt * 128 : (dt + 1) * 128],
                        et[kt][:, qc * 512 : (qc + 1) * 512],
                        start=(kt == 0),
                        stop=(kt == KT - 1),
                    )
                if ch == 0:
                    for qc2 in range(2):
                        nc.vector.reciprocal(
                            out=recip[:, qc2 * 512 : (qc2 + 1) * 512],
                            in_=rowacc[:, qc2 * 512 : (qc2 + 1) * 512],
                        )
                nc.vector.scalar_tensor_tensor(
                    out=ct[dt][:, qc * 512 : (qc + 1) * 512],
                    in0=c_ps,
                    scalar=0.0,
                    in1=recip[:, qc * 512 : (qc + 1) * 512],
                    op0=mybir.AluOpType.bypass,
                    op1=mybir.AluOpType.mult,
                )

            # ---- Phase D: out^T = Wv C^T ----
            for ch in range(15):
                ot, qc = divmod(ch, 2)
                d_ps = pp.tile([128, 512], F32, tag="ps", name=f"dps{sfx}_{ch}")
                for dk in range(DK):
                    nc.tensor.matmul(
                        d_ps,
                        wv[dk][:, ot * 128 : (ot + 1) * 128],
                        ct[dk][:, qc * 512 : (qc + 1) * 512],
                        start=(dk == 0),
                        stop=(dk == DK - 1),
                    )
                oev = sb.tile(
                    [128, 512], F32, tag="oev", bufs=3, name=f"oev{sfx}_{ch}"
                )
                nc.vector.tensor_copy(out=oev, in_=d_ps)
                dma_eng = nc.sync if ch % 2 == 0 else nc.gpsimd
                dma_eng.dma_start(
                    out=out_d[ot * 128 : (ot + 1) * 128, qc * 512 : (qc + 1) * 512],
                    in_=oev,
                )
            # Final (ot7, qc1) chain as 4 [128,128] sub-chains so the tail
            # is one narrow evict+DMA instead of a full 512-wide one.
            for c4 in range(4):
                f_ps = pp.tile([128, 512], F32, tag="ps", name=f"fps{sfx}_{c4}")[
                    :, 0:128
                ]
                lo = 512 + c4 * 128
                for dk in range(DK):
                    nc.tensor.matmul(
                        f_ps,
                        wv[dk][:, 7 * 128 : 8 * 128],
                        ct[dk][:, lo : lo + 128],
                        start=(dk == 0),
                        stop=(dk == DK - 1),
                    )
                fev = sb.tile([128, 128], F32, tag="fev", bufs=4, name=f"fev{sfx}_{c4}")
                if c4 % 2 == 0:
                    nc.vector.tensor_copy(out=fev, in_=f_ps)
                else:
                    nc.scalar.copy(out=fev, in_=f_ps)
                dma_eng = nc.gpsimd if c4 % 2 == 0 else nc.sync
                dma_eng.dma_start(
                    out=out_d[7 * 128 : 8 * 128, lo : lo + 128], in_=fev
                )
    return nc


def _get_program():
    if "nc" not in _CACHE:
        nc = bacc.Bacc("TRN2", target_bir_lowering=False, num_devices=N_CORES)
        _emit(nc)
        nc.compile()
        _CACHE["nc"] = nc
    return _CACHE["nc"]


def kernel(x, Wq, Wk, Wv):
    bf = ml_dtypes.bfloat16
    x = np.asarray(x, dtype=np.float32)
    Wq = np.asarray(Wq, dtype=np.float32)
    Wk = np.asarray(Wk, dtype=np.float32)
    Wv = np.asarray(Wv, dtype=np.float32)

    nc = _get_program()
    m = np.ascontiguousarray(Wq.T @ Wk).astype(bf)  # M = Wq^T Wk, [d1, d2]
    wvt = np.ascontiguousarray(Wv.T).astype(bf)  # [D, O]
    in_maps = []
    for c in range(N_CORES):
        b, h = divmod(c, 2)
        xp = np.concatenate(
            [x[b, h * HQ : (h + 1) * HQ], x[b, (1 - h) * HQ : (2 - h) * HQ]], axis=0
        )
        in_maps.append(
            {
                "xt": np.ascontiguousarray(xp.T).astype(bf),
                "xn": xp.astype(bf),
                "m": m,
                "wvt": wvt,
            }
        )
    res = run_bass_kernel_spmd(nc, in_maps, list(range(N_CORES)))
    outp = np.empty((B, S, O), dtype=np.float32)
    for c in range(N_CORES):
        b, h = divmod(c, 2)
        outp[b, h * HQ : (h + 1) * HQ] = res.results[c]["outT"].T
    return outp
